# revision 1
# baseline (speedup 1.0000x reference)
"""Multi-head attention (RoPE + masked softmax) on 8 TRN2 NeuronCores.

Sharding per the tensor-parallel hint: 2 batch groups x 4 cores; each core
computes 4 of the 16 heads (q/k/v projection column slices) for the full
sequence of its batch, then its partial output projection, and a per-window
ReduceScatter over the 4-core group sums the o-proj partials while handing
each core one seq sub-shard per window (host reassembles).

The attention mask is classified host-side into 128x128 blocks
(skip / identity / mixed) shared across both batches, and the program is
compiled specialized to that block map: fully-masked blocks are never
computed, identity blocks skip the mask multiply, and mixed blocks multiply
a per-block exp(mask) tile. Causal masks dedupe to a single mixed tile.
"""

from contextlib import ExitStack

import numpy as np

import concourse.bass as bass
import concourse.tile as tile
from concourse import bacc, mybir
from concourse.alu_op_type import AluOpType
from concourse.bass_utils import run_bass_kernel_spmd

AF = mybir.ActivationFunctionType
F32 = mybir.dt.float32
F32R = mybir.dt.float32r
F16 = mybir.dt.float16
BF16 = mybir.dt.bfloat16

B, S, HID, NH, HD = 2, 2048, 1024, 16, 64
SCALE = 1.0 / np.sqrt(HD)
N_CORES = 8
HPC = 4          # heads per core
HC = HID // 128  # hidden 128-chunks (8)
KC = S // 128    # key 128-chunks (16)
QB = S // 512    # query windows of 512 (4)

SKIP = -1
ONES = 0


def _runs(labels):
    """Maximal runs of non-skip entries: [(start, end)]."""
    out = []
    i = 0
    n = len(labels)
    while i < n:
        if labels[i] == SKIP:
            i += 1
            continue
        j = i
        while j < n and labels[j] != SKIP:
            j += 1
        out.append((i, j))
        i = j
    return out


def build_program(cls_map):
    """cls_map[kc][qc] over the 16x16 grid of (key, query) 128x128 blocks:
    SKIP (-1), ONES (0), or 1-based mixed-tile index."""
    n_mixed = max((v for row in cls_map for v in row if v > 0), default=0)

    nc = bacc.Bacc("TRN2", target_bir_lowering=False, debug=False,
                   num_devices=N_CORES)

    hsT = nc.dram_tensor("hsT", [HID, S], BF16, kind="ExternalInput").ap()
    cosk = nc.dram_tensor("cosk", [128, S], BF16, kind="ExternalInput").ap()
    sink = nc.dram_tensor("sink", [128, S], BF16, kind="ExternalInput").ap()
    wq = nc.dram_tensor("wq", [HID, 256], BF16, kind="ExternalInput").ap()
    wk = nc.dram_tensor("wk", [HID, 256], BF16, kind="ExternalInput").ap()
    wv = nc.dram_tensor("wv", [HID, 256], BF16, kind="ExternalInput").ap()
    wo = nc.dram_tensor("wo", [HID, HID], BF16, kind="ExternalInput").ap()
    emt = nc.dram_tensor("emt", [128, max(n_mixed, 1) * 128], BF16,
                         kind="ExternalInput").ap()
    sel4 = nc.dram_tensor("sel4", [4, 256], F32R, kind="ExternalInput").ap()
    roff_t = nc.dram_tensor("roff", [1, 1], mybir.dt.uint32,
                            kind="ExternalInput").ap()
    out = nc.dram_tensor("out", [QB * 128, HID], F16, kind="ExternalOutput").ap()

    with tile.TileContext(nc) as tc, ExitStack() as top:
        res = top.enter_context(tc.tile_pool(name="res", bufs=1))
        dram = top.enter_context(tc.tile_pool(name="dram", bufs=1, space="DRAM"))

        # ---- resident tiles + input DMAs ---------------------------------
        hsT_sb = []
        for hc in range(HC):
            t = res.tile([128, S], BF16, tag=f"hsT{hc}", name=f"hsT{hc}")
            nc.sync.dma_start(t[:], hsT[hc * 128:(hc + 1) * 128, :])
            hsT_sb.append(t)
        wq_sb, wk_sb, wv_sb = [], [], []
        for nm, src, dst in (("wk", wk, wk_sb), ("wq", wq, wq_sb),
                             ("wv", wv, wv_sb)):
            for hc in range(HC):
                t = res.tile([128, 256], BF16, tag=f"{nm}r{hc}",
                             name=f"{nm}r{hc}")
                nc.scalar.dma_start(t[:], src[hc * 128:(hc + 1) * 128, :])
                dst.append(t)
        cos_sb = res.tile([128, S], BF16, tag="cos")
        nc.sync.dma_start(cos_sb[:], cosk[:])
        sin_sb = res.tile([128, S], BF16, tag="sin")
        nc.sync.dma_start(sin_sb[:], sink[:])
        emt_sb = res.tile([128, max(n_mixed, 1) * 128], BF16, tag="emt")
        nc.sync.dma_start(emt_sb[:], emt[:])
        wo_sb = []
        for hc in range(HC):
            t = res.tile([128, HID], BF16, tag=f"wo{hc}", name=f"wo{hc}")
            nc.scalar.dma_start(t[:], wo[hc * 128:(hc + 1) * 128, :])
            wo_sb.append(t)

        # V_aug: per key-chunk, 4 heads x (64 cols + ones col)
        v_sb = [res.tile([128, HPC * 65], BF16, tag=f"v{kc}", name=f"v{kc}")
                for kc in range(KC)]
        for kc in range(KC):
            v3 = v_sb[kc][:].rearrange("p (h c) -> p h c", h=HPC)
            nc.gpsimd.memset(v3[:, :, 64], 1.0)
        kt_sb = [res.tile([128, S], BF16, tag=f"kt{p}", name=f"kt{p}")
                 for p in range(2)]
        qt_sb = [res.tile([128, S], BF16, tag=f"qt{p}", name=f"qt{p}")
                 for p in range(2)]
        acc_sb = [res.tile([128, S], BF16, tag=f"acc{p}", name=f"acc{p}")
                  for p in range(2)]
        acc2 = [res.tile([128, S], BF16, tag=f"acc2_{p}", name=f"acc2_{p}")
                for p in range(2)]
        den_all = res.tile([4, S], F32, tag="den")
        recip_all = res.tile([4, S], F32R, tag="recip")
        sel_sb = res.tile([4, 256], F32R, tag="sel")
        nc.sync.dma_start(sel_sb[:], sel4[:])

        # DRAM bounce buffers for the per-window AllGather of attn outputs
        ag_wu_in = dram.tile([8, 8], BF16, name="agwui")
        ag_wu_out = dram.tile([32, 8], BF16, name="agwuo")
        ag_in = [dram.tile([256, 512], BF16, name=f"agi{w}") for w in range(QB)]
        ag_out = [dram.tile([1024, 512], BF16, name=f"ago{w}")
                  for w in range(QB)]

        # ---- Q/K/V projections + RoPE ------------------------------------
        def rope_apply(ropep, dst_tile, ps, s0):
            """dst[:, s0:s0+512] = rope(ps); tables pre-gathered/shifted."""
            with nc.allow_low_precision(reason="bf16 rope"):
                kraw = ropep.tile([128, 512], BF16, tag="kraw")
                nc.scalar.copy(kraw[:], ps[:])
                t1 = ropep.tile([128, 512], BF16, tag="t1")
                nc.vector.tensor_tensor(
                    t1[:], kraw[:], cos_sb[:, s0:s0 + 512], AluOpType.mult)
                t2 = ropep.tile([128, 512], BF16, tag="t2")
                for i, hb in enumerate((0, 64)):
                    eng = nc.vector if i == 0 else nc.gpsimd
                    eng.tensor_tensor(
                        t2[hb:hb + 32, :], kraw[hb + 32:hb + 64, :],
                        sin_sb[hb + 32:hb + 64, s0:s0 + 512], AluOpType.mult)
                    eng.tensor_tensor(
                        t2[hb + 32:hb + 64, :], kraw[hb:hb + 32, :],
                        sin_sb[hb:hb + 32, s0:s0 + 512], AluOpType.mult)
                nc.vector.tensor_tensor(
                    dst_tile[:, s0:s0 + 512], t1[:], t2[:], AluOpType.add)

        pss = top.enter_context(tc.tile_pool(name="pss", bufs=2, space="PSUM"))
        psa = top.enter_context(tc.tile_pool(name="psa", bufs=2, space="PSUM"))
        pbo = top.enter_context(tc.tile_pool(name="pbo", bufs=2, space="PSUM"))
        pp = top.enter_context(tc.tile_pool(name="pp", bufs=4))
        outp = top.enter_context(tc.tile_pool(name="outp", bufs=3))

        ropep = top.enter_context(tc.tile_pool(name="ropep", bufs=3))

        def emit_proj_all():
            nc.gpsimd.collective_compute(
                "AllGather", AluOpType.bypass,
                replica_groups=[[0, 1, 2, 3], [4, 5, 6, 7]],
                ins=[ag_wu_in[:].opt()], outs=[ag_wu_out[:].opt()])
            def emit_v(kc):
                psf = pbo.tile([128, 512], F32, tag="pbo", name="psvf")
                ps = psf[:, 0:256]
                for hc in range(HC):
                    nc.tensor.matmul(
                        ps, hsT_sb[hc][:, kc * 128:(kc + 1) * 128],
                        wv_sb[hc][:], start=(hc == 0), stop=(hc == HC - 1))
                v3 = v_sb[kc][:].rearrange("p (h c) -> p h c", h=HPC)
                ps3 = ps.rearrange("p (h c) -> p h c", h=HPC)
                with nc.allow_low_precision(reason="bf16 V"):
                    nc.scalar.copy(v3[:, :, 0:64], ps3[:])

            # K/Q chunk tiles interleaved with V-projection psums so the PE
            # fills RoPE-induced stalls; both pairs' low seq chunks first so
            # window 0 unblocks early.
            vq = 0
            for sc2 in range(QB // 2):
                for w_sb, dst in ((wk_sb, kt_sb), (wq_sb, qt_sb)):
                    for p in range(2):
                        c0 = p * 128
                        ps2 = pss.tile([128, 1024], F32, tag="pss",
                                       name="pskq")
                        for j in range(2):
                            sc = 2 * sc2 + j
                            ps = ps2[:, j * 512:(j + 1) * 512]
                            for hc in range(HC):
                                nc.tensor.matmul(
                                    ps, w_sb[hc][:, c0:c0 + 128],
                                    hsT_sb[hc][:, sc * 512:(sc + 1) * 512],
                                    start=(hc == 0), stop=(hc == HC - 1))
                            rope_apply(ropep, dst[p], ps, sc * 512)
                        emit_v(vq)
                        emit_v(vq + 1)
                        vq += 2

        # ---- attention + per-window normalize/o-proj/ReduceScatter -------
        # item = (h, qb, kcs(list), runs(per kc), start, stop)
        by_window = []
        for qb in range(QB):
            items = []
            for h in range(HPC):
                parts = [kc for kc in range(KC)
                         if any(cls_map[kc][4 * qb + j] != SKIP
                                for j in range(4))]
                assert parts, f"no participating key blocks for window {qb}"
                for i2 in range(0, len(parts), 2):
                    kcs = parts[i2:i2 + 2]
                    runs = [_runs([cls_map[kc][4 * qb + j] for j in range(4)])
                            for kc in kcs]
                    items.append([h, qb, kcs, runs,
                                  i2 == 0, i2 + 2 >= len(parts)])
            by_window.append(items)
        flat = [it for w in by_window for it in w]

        if True:
            _rreg = nc.gpsimd.alloc_register("roff_reg")
            nc.gpsimd.reg_load(_rreg, roff_t[0:1, 0:1])
            roff = nc.gpsimd.snap(_rreg, donate=True, min_val=0, max_val=384)
            psa_t = {}
            ps_of_item = {}

            def emit_scores(i):
                h, qb, kcs, runs, start, stop = flat[i]
                p, hb = h // 2, (h % 2) * 64
                ps_s = pss.tile([128, 1024], F32, tag="pss")
                for j, kc in enumerate(kcs):
                    for r0, r1 in runs[j]:
                        nc.tensor.matmul(
                            ps_s[:, j * 512 + r0 * 128: j * 512 + r1 * 128],
                            kt_sb[p][hb:hb + 64, kc * 128:(kc + 1) * 128],
                            qt_sb[p][hb:hb + 64,
                                     qb * 512 + r0 * 128: qb * 512 + r1 * 128],
                            start=True, stop=True)
                ps_of_item[i] = ps_s

            def emit_expmask(i):
                h, qb, kcs, runs, start, stop = flat[i]
                ps_s = ps_of_item.pop(i)
                P = pp.tile([128, 1024], BF16, tag="pp")
                if len(kcs) == 2 and runs[0] == runs[1] == [(0, 4)]:
                    nc.scalar.activation(P[:], ps_s[:], AF.Exp)
                else:
                    for j in range(len(kcs)):
                        for r0, r1 in runs[j]:
                            nc.scalar.activation(
                                P[:, j * 512 + r0 * 128: j * 512 + r1 * 128],
                                ps_s[:, j * 512 + r0 * 128: j * 512 + r1 * 128],
                                AF.Exp)
                for j, kc in enumerate(kcs):
                    labels = [cls_map[kc][4 * qb + jj] for jj in range(4)]
                    if start and j == 0 and runs[j] != [(0, 4)]:
                        # first accumulation must initialize all 512 cols
                        zr = [jj for jj in range(4) if labels[jj] == SKIP]
                        jj = 0
                        while jj < len(zr):
                            j2 = jj
                            while j2 + 1 < len(zr) and zr[j2 + 1] == zr[j2] + 1:
                                j2 += 1
                            nc.vector.memset(
                                P[:, j * 512 + zr[jj] * 128:
                                  j * 512 + (j2 + 1 and (zr[j2] + 1)) * 128],
                                0.0)
                            jj = j2 + 1
                    for jj in range(4):
                        mix = labels[jj]
                        if mix > 0:
                            sl = P[:, j * 512 + jj * 128: j * 512 + (jj + 1) * 128]
                            with nc.allow_low_precision(reason="bf16 mask"):
                                nc.gpsimd.tensor_tensor(
                                    sl, sl,
                                    emt_sb[:, (mix - 1) * 128: mix * 128],
                                    AluOpType.mult)
                return P

            def emit_attnv(i, P):
                h, qb, kcs, runs, start, stop = flat[i]
                if start:
                    psa_t[(h, qb)] = psa.tile([128, 512], F32, tag="psa",
                                              name="psat")
                pa = psa_t[(h, qb)]
                for j, kc in enumerate(kcs):
                    first = start and j == 0
                    last = stop and j == len(kcs) - 1
                    if first:
                        nc.tensor.matmul(
                            pa[0:65, :], v_sb[kc][:, h * 65: h * 65 + 65],
                            P[:, j * 512:(j + 1) * 512],
                            start=True, stop=last)
                    else:
                        for r0, r1 in runs[j]:
                            nc.tensor.matmul(
                                pa[0:65, r0 * 128:r1 * 128],
                                v_sb[kc][:, h * 65: h * 65 + 65],
                                P[:, j * 512 + r0 * 128: j * 512 + r1 * 128],
                                start=False,
                                stop=last and (r0, r1) == runs[j][-1])
                if stop:
                    p, hb = h // 2, (h % 2) * 64
                    dtmp = pp.tile([1, 512], F32, tag="dtmp", name="dtmp")
                    nc.vector.tensor_copy(dtmp[:], pa[64:65, :])
                    nc.sync.dma_start(
                        den_all[h:h + 1, qb * 512:(qb + 1) * 512], dtmp[:])
                    with nc.allow_low_precision(reason="bf16 attn accum"):
                        nc.vector.tensor_copy(
                            acc_sb[p][hb:hb + 64, qb * 512:(qb + 1) * 512],
                            pa[0:64, :])
                    del psa_t[(h, qb)]

            def emit_window_tail(w):
                with nc.allow_low_precision(reason="f32r reciprocal"):
                    nc.vector.reciprocal(recip_all[:, w * 512:(w + 1) * 512],
                                         den_all[:, w * 512:(w + 1) * 512])
                for p in range(2):
                    ps_bc = pbo.tile([128, 512], F32, tag="pbo",
                                     name="psbc")
                    nc.tensor.matmul(ps_bc[:],
                                     sel_sb[:, p * 128:(p + 1) * 128],
                                     recip_all[:, w * 512:(w + 1) * 512],
                                     start=True, stop=True)
                    with nc.allow_low_precision(reason="bf16 attn weights"):
                        nc.vector.tensor_tensor(
                            acc2[p][:, w * 512:(w + 1) * 512],
                            acc_sb[p][:, w * 512:(w + 1) * 512], ps_bc[:],
                            AluOpType.mult)
                for p in range(2):
                    nc.gpsimd.dma_start(
                        ag_in[w][p * 128:(p + 1) * 128, :],
                        acc2[p][:, w * 512:(w + 1) * 512])
                nc.gpsimd.collective_compute(
                    "AllGather", AluOpType.bypass,
                    replica_groups=[[0, 1, 2, 3], [4, 5, 6, 7]],
                    ins=[ag_in[w][:].opt()],
                    outs=[ag_out[w][:].opt()])

            def emit_tail_b(w):
                aot = outp.tile([128, HC * 128], BF16, tag="aot",
                                name="aot")
                ao3 = aot[:].rearrange("b (a c) -> b a c", a=HC)
                src = ag_out[w][:].rearrange("(a b) c -> b a c", b=128)
                nc.gpsimd.dma_start(ao3[:, :, :],
                                    src[:, :, bass.ds(roff, 128)])
                for nn in range(2):
                    ps = pbo.tile([128, 512], F32, tag="pbo", name="pso")
                    for kk in range(HC):
                        nc.tensor.matmul(
                            ps[:], ao3[:, kk, :],
                            wo_sb[kk][:, nn * 512:(nn + 1) * 512],
                            start=(kk == 0), stop=(kk == HC - 1))
                    t_o = outp.tile([128, 512], F16, tag="tout")
                    with nc.allow_low_precision(reason="f16 out"):
                        nc.vector.tensor_copy(t_o[:], ps[:])
                    nc.sync.dma_start(
                        out[w * 128:(w + 1) * 128,
                            nn * 512:(nn + 1) * 512], t_o[:])

            # software pipeline: scores one item ahead of exp+attnV;
            # window tails interleave between windows; second projection
            # half is emitted between windows 1 and 2.
            emit_proj_all()
            emit_scores(0)
            gi = 0
            for w, items in enumerate(by_window):
                for _ in items:
                    if gi + 1 < len(flat):
                        emit_scores(gi + 1)
                    P = emit_expmask(gi)
                    emit_attnv(gi, P)
                    gi += 1
                emit_window_tail(w)
                if w >= 1:
                    emit_tail_b(w - 1)
            emit_tail_b(QB - 1)

    nc.compile()
    return nc


_PROGRAM_CACHE = {}


def _classify_mask(attention_mask):
    """Shared 16x16 block classification + per-batch mixed tile data."""
    m = np.asarray(attention_mask, np.float32)  # [B, 1, S, S]
    cls = [[ONES] * KC for _ in range(KC)]
    tiles = {}
    tile_data = [[], []]
    for kc in range(KC):
        for qc in range(KC):
            subs = [m[b, 0, qc * 128:(qc + 1) * 128,
                      kc * 128:(kc + 1) * 128] for b in range(B)]
            if all(np.all(s < -30.0) for s in subs):
                cls[kc][qc] = SKIP
            elif all(np.all(np.abs(s) < 1e-6) for s in subs):
                cls[kc][qc] = ONES
            else:
                es = [np.exp(np.minimum(s.T, 30.0)).astype(np.float32)
                      for s in subs]
                key = tuple(e.tobytes() for e in es)
                if key not in tiles:
                    tiles[key] = len(tiles) + 1
                    tile_data[0].append(es[0])
                    tile_data[1].append(es[1])
                cls[kc][qc] = tiles[key]
    return cls, tile_data


def _get_program(cls_map):
    key = tuple(tuple(r) for r in cls_map)
    if key not in _PROGRAM_CACHE:
        _PROGRAM_CACHE[key] = build_program(cls_map)
    return _PROGRAM_CACHE[key]


def make_in_maps(hidden_states, attention_mask, position_ids, cos, sin,
                 Wq, Wk, Wv, Wo):
    import ml_dtypes
    bf16 = ml_dtypes.bfloat16
    hidden_states = np.asarray(hidden_states, np.float32)
    position_ids = np.asarray(position_ids)
    cos = np.asarray(cos, np.float32)
    sin = np.asarray(sin, np.float32)
    Wq = np.asarray(Wq, np.float32) * SCALE
    Wk = np.asarray(Wk, np.float32)
    Wv = np.asarray(Wv, np.float32)
    Wo = np.asarray(Wo, np.float32)

    cls_map, tile_data = _classify_mask(attention_mask)

    sel = np.zeros((4, 256), np.float32)
    for p in range(2):
        for mm in range(128):
            sel[2 * p + (mm >= 64), 128 * p + mm] = 1.0

    wo_full = np.ascontiguousarray(Wo).astype(bf16)
    per_batch = []
    for b in range(B):
        hsT_b = np.ascontiguousarray(hidden_states[b].T).astype(bf16)
        cos_b = cos[position_ids[b]]  # [S, 64]
        sin_b = sin[position_ids[b]]
        cosT = np.tile(cos_b.T, (2, 1)).astype(bf16)  # [128, S]
        sin64 = sin_b.T
        sh = np.empty_like(sin64)
        sh[0:32] = sin64[32:64]
        sh[32:64] = -sin64[0:32]
        sinT = np.tile(sh, (2, 1)).astype(bf16)
        if tile_data[b]:
            emt_b = np.concatenate(tile_data[b], axis=1).astype(bf16)
        else:
            emt_b = np.ones((128, 128), np.float32).astype(bf16)
        per_batch.append((hsT_b, cosT, sinT, np.ascontiguousarray(emt_b)))

    in_maps = []
    for c in range(N_CORES):
        g, r = c // 4, c % 4
        hsT_b, cosT, sinT, emt_b = per_batch[g]
        in_maps.append({
            "hsT": hsT_b, "cosk": cosT, "sink": sinT, "emt": emt_b,
            "wq": np.ascontiguousarray(Wq[:, 256 * r:256 * (r + 1)]).astype(bf16),
            "wk": np.ascontiguousarray(Wk[:, 256 * r:256 * (r + 1)]).astype(bf16),
            "wv": np.ascontiguousarray(Wv[:, 256 * r:256 * (r + 1)]).astype(bf16),
            "wo": wo_full, "sel4": sel,
            "roff": np.array([[r * 128]], np.uint32),
        })
    return in_maps, cls_map


def run(inputs: dict, trace: bool = False):
    in_maps, cls_map = make_in_maps(**inputs)
    nc = _get_program(cls_map)
    res = run_bass_kernel_spmd(nc, in_maps, list(range(N_CORES)), trace=trace)
    out = np.empty((B, S, HID), np.float32)
    for c in range(N_CORES):
        g, r = c // 4, c % 4
        o = np.asarray(res.results[c]["out"], np.float32)  # [512, 1024]
        for w in range(QB):
            s0 = w * 512 + r * 128
            out[g, s0:s0 + 128, :] = o[w * 128:(w + 1) * 128, :]
    return out, res


def kernel(**inputs) -> np.ndarray:
    out, _ = run(inputs, trace=False)
    return out

